# revision 45
# baseline (speedup 1.0000x reference)
"""Trainium2 Bass kernel for nn_NeuralDecisionTree.

Strategy (data-parallel over batch, 8 cores):
  reference:  x = features @ mask.T            [B, 1024]   (one-hot row select)
              d = sigmoid(x @ W + b)           [B, 1024]
              mu = tree-routing products       [B, 1024]
              out = mu @ softmax(pi)           [B, 100]

  The mask matmul is an exact column-selection, folded into W on the host.
  The host also pre-transposes/gathers features into [feature, batch] chunk
  layout and quantizes both matmul operands to fp8 e4m3 (W scaled by 16,
  descaled inside the fused sigmoid), so the device main matmul runs in
  DoubleRow fp8 mode: each MM contracts 256 features (two 128-row chunks
  packed per PE cell) at ~2x bf16 throughput.

    zT[s, b]  = sum_f W2p[f, s] * featT[f, b]             (PE, fp8 DoubleRow)
    d         = sigmoid(zT/16 + b)                        (ACT, bf16 out)
    mu        = 10 levels of routing products             (DVE, bf16 2x mode;
                right child = mu - mu*d, so no second sigmoid is needed)
    yT[c, b]  = sum_s probsP[s, c] * mu10[s, b]           (PE, bf16)

  Node outputs are permuted on the host (slot permutation) so every tree
  level consumes a contiguous slice of d; levels 0-6 run in [batch, path]
  layout, levels 7-9 in [path-partition, batch] layout, and the leaf order
  is absorbed into a host-side row permutation of pi.  The two in-SBUF
  transposes (d-tile0 and mu7) run on the DMA xbar in bf16, keeping the PE
  stream pure matmul.
"""

import ml_dtypes
import numpy as np

import concourse.bass as bass  # noqa: F401
import concourse.mybir as mybir
import concourse.tile as tile
from concourse import bacc
from concourse.bass_utils import run_bass_kernel_spmd

F32 = mybir.dt.float32
BF16 = mybir.dt.bfloat16
FP8 = mybir.dt.float8e4

B = 16384
NCORES = 8
BC = B // NCORES      # 2048 batch rows per core
SG = 512              # batch rows processed end-to-end per stage
NSG = BC // SG        # 4
NF = 1024             # used features (host gathers mask-selected columns)
NL = 1024             # tree nodes / leaves / dense units
NCLS = 100            # classes
KCH = NF // 128       # 8 contraction chunks of 128
NDR = KCH // 2        # 4 double-row chunks of 256
NT = NL // 128        # 8 slot tiles
WSCALE = 16.0         # host premultiplies W2 by this; sigmoid descales

# test.py can override (e.g. {"trace": True}) and read LAST_RESULT
RUN_KWARGS: dict = {}
LAST_RESULT = None


def _bitrev(q: int, bits: int) -> int:
    r = 0
    for m in range(bits):
        if (q >> m) & 1:
            r |= 1 << (bits - 1 - m)
    return r


def _node_of_slot() -> np.ndarray:
    """slot -> original node id. Slots are laid out so each tree level reads
    a contiguous [128, SG] slice of d at aligned partitions."""
    node = np.zeros(NL, dtype=np.int64)
    node[0] = 0  # unused slot (level-l nodes live at slots [2^l, 2^(l+1)),
    # so every phase-A slice starts at an even, 4B-aligned bf16 offset)
    for l in range(7):
        for q in range(1 << l):
            node[(1 << l) + q] = (1 << l) + _bitrev(q, l)
    for q7 in range(128):
        node[128 + q7] = 128 + _bitrev(q7, 7)
    for j1 in range(2):
        for q7 in range(128):
            node[256 + j1 * 128 + q7] = 256 + 2 * _bitrev(q7, 7) + j1
    for j2 in range(4):
        c7, c8 = j2 & 1, j2 >> 1
        for q7 in range(128):
            node[512 + j2 * 128 + q7] = 512 + 4 * _bitrev(q7, 7) + 2 * c7 + c8
    return node


def _leaf_of_row() -> np.ndarray:
    """probsP row r = j3*128 + q7 -> original leaf index."""
    L = np.zeros(NL, dtype=np.int64)
    for j3 in range(8):
        c789 = [j3 & 1, (j3 >> 1) & 1, (j3 >> 2) & 1]
        for q7 in range(128):
            c = [(q7 >> m) & 1 for m in range(7)] + c789
            L[j3 * 128 + q7] = sum(c[m] << (9 - m) for m in range(10))
    return L


def _build_program():
    nc = bacc.Bacc("TRN2", target_bir_lowering=False)
    feat = nc.dram_tensor("feat", [128, NSG * KCH * SG], FP8, kind="ExternalInput")
    w2p = nc.dram_tensor("w2p", [128, NT * NF], FP8, kind="ExternalInput")
    biases = nc.dram_tensor("biases", [128, 2 * NT], F32, kind="ExternalInput")
    pip = nc.dram_tensor("pip", [128, NT * NCLS], BF16, kind="ExternalInput")
    yT = nc.dram_tensor("yT", [NCLS, BC], F32, kind="ExternalOutput")

    SIG = mybir.ActivationFunctionType.Sigmoid
    DR = mybir.MatmulPerfMode.DoubleRow
    SGB = KCH * SG  # fp8 bytes per sg slice of feat, per partition

    with tile.TileContext(nc) as tc:
        with (
            tc.tile_pool(name="const", bufs=1) as cpool,
            tc.tile_pool(name="featT", bufs=2) as ftpool,
            tc.tile_pool(name="dsig", bufs=2) as dpool,
            tc.tile_pool(name="tree", bufs=2) as tpool,
            tc.tile_pool(name="mu", bufs=1) as mupool,
            tc.tile_pool(name="outst", bufs=2) as opool,
            tc.tile_pool(name="pw", bufs=1, space="PSUM") as pw,
            tc.tile_pool(name="pz", bufs=4, space="PSUM") as pz,
            tc.tile_pool(name="py", bufs=2, space="PSUM") as py,
        ):
            QB = 2 * SG  # fp8 bytes per DR-chunk quarter, per partition

            def load_ft(sg):
                """One tile per DoubleRow chunk so the first matmuls only
                depend on their own quarter's DMA."""
                fq = []
                for c in range(NDR):
                    q = ftpool.tile([128, QB], FP8, tag=f"ft{c}")
                    nc.sync.dma_start(
                        q, feat[:, sg * SGB + c * QB:sg * SGB + (c + 1) * QB]
                    )
                    fq.append(q)
                return fq

            # ---- sg0 features + tile-0 weights lead the DMA queue (they
            # gate the first matmul); bulk weights follow as one burst ----
            ft0q0 = ftpool.tile([128, QB], FP8, tag="ft0")
            nc.sync.dma_start(ft0q0, feat[:, 0:QB])
            w2 = cpool.tile([128, NT * NF], FP8)
            nc.sync.dma_start(w2[:, 0:NF], w2p[:, 0:NF])
            bia = cpool.tile([128, 2 * NT], F32)
            nc.sync.dma_start(bia, biases[:, :])
            ft0 = [ft0q0]
            for c in range(1, NDR):
                q = ftpool.tile([128, QB], FP8, tag=f"ft{c}")
                nc.sync.dma_start(q, feat[:, c * QB:(c + 1) * QB])
                ft0.append(q)
            for t in range(1, NT):
                nc.sync.dma_start(
                    w2[:, t * NF:(t + 1) * NF], w2p[:, t * NF:(t + 1) * NF]
                )
            pp = cpool.tile([128, NT * NCLS], BF16)
            nc.sync.dma_start(pp, pip[:, :])

            # warm-up burst: keep the PE busy during the head DMA wait so the
            # HAM clock gate is at 8/8 when the first real matmuls issue.
            wt = cpool.tile([128, 128], BF16)
            nc.gpsimd.memset(wt, 0.0)
            wp = pw.tile([128, 128], F32, tag="pt")
            for _ in range(26):
                nc.tensor.matmul(wp, wt, wt, start=True, stop=True)

            ones = cpool.tile([128, 4], BF16)
            nc.gpsimd.memset(ones, 1.0)
            ones3 = ones.rearrange("p (u w) -> p u w", u=4)

            def transpose_mu7(mu7):
                """mu7 -> [path-part, batch] via DMA xbar.  Issued on the SP
                queue right after the next sg's feature loads, so it neither
                blocks them nor races the next t0T (which becomes ready
                later than this does)."""
                m7T = tpool.tile([128, 512], BF16, tag="m7T")
                nc.sync.dma_start_transpose(
                    m7T.rearrange("p (u q) -> p u q", u=4), mu7
                )
                return m7T

            def stage1(sg, ft):
                """fp8 MM block + sigmoids + t0 DMA transpose + tree A."""

                d0 = dpool.tile([128, SG], BF16, tag="d0")
                dsg = dpool.tile([128, 7 * SG], BF16, tag="d")
                t0T = tpool.tile([128, 512], BF16, tag="t0T")

                scope_mm = nc.named_scope(f"mm{sg}")
                scope_mm.__enter__()
                for t in range(NT):
                    zp = pz.tile([128, SG], F32, tag="z")
                    for c in range(NDR):
                        wsl = w2[:, (t * KCH + 2 * c) * 128:
                                 (t * KCH + 2 * c + 2) * 128]
                        nc.tensor.matmul(
                            zp,
                            wsl.rearrange("p (k s) -> p k s", k=2),
                            ft[c].rearrange("p (k b) -> p k b", k=2),
                            start=(c == 0), stop=(c == NDR - 1),
                            perf_mode=DR,
                        )
                    dst = d0 if t == 0 else dsg[:, (t - 1) * SG:t * SG]
                    nc.scalar.activation(
                        dst, zp, SIG, bias=bia[:, t:t + 1], scale=1.0 / WSCALE
                    )
                    if t == 0:
                        # d-tile0 -> [batch-part, slot] via DMA xbar
                        nc.sync.dma_start_transpose(
                            t0T.rearrange("p (u s) -> p u s", u=4), d0
                        )

                scope_mm.__exit__(None, None, None)
                # tree phase A (levels 0-6) in [b, path] layout, bf16
                scope_pa = nc.named_scope(f"pA{sg}")
                scope_pa.__enter__()
                t03 = t0T.rearrange("p (u w) -> p u w", u=4)
                mu_prev = mupool.tile([128, 4 * 2], BF16, tag="muA1", bufs=2)
                mp3 = mu_prev.rearrange("p (u w) -> p u w", u=4)
                nc.vector.tensor_copy(mp3[:, :, 0:1], t03[:, :, 1:2])
                nc.vector.tensor_sub(mp3[:, :, 1:2], ones3, t03[:, :, 1:2])
                for l in range(1, 7):
                    w = 1 << l
                    mu_next = mupool.tile(
                        [128, 4 * 2 * w], BF16, tag=f"muA{l + 1}", bufs=2
                    )
                    mn3 = mu_next.rearrange("p (u w) -> p u w", u=4)
                    nc.vector.tensor_mul(
                        mn3[:, :, 0:w], mp3, t03[:, :, w:2 * w]
                    )
                    nc.vector.tensor_sub(mn3[:, :, w:2 * w], mp3, mn3[:, :, 0:w])
                    mu_prev, mp3 = mu_next, mn3

                # mu7 transpose at the end of its own stage1: with features
                # prefetched an iteration ahead, the SP queue order per sg is
                # [ft(sg+1), t0T(sg), m7T(sg)] — monotone in ready-time, so
                # no trigger ever blocks a sooner-needed one.
                m7T = transpose_mu7(mu_prev)
                scope_pa.__exit__(None, None, None)
                return sg, dsg, m7T

            def stage2(state):
                """tree phase B + leaf matmul + output DMA."""
                sg, dsg, m7T = state
                scope_pb = nc.named_scope(f"pB{sg}")
                scope_pb.__enter__()
                mu8 = mupool.tile([128, 2 * SG], BF16, tag="mu8")
                nc.vector.tensor_mul(mu8[:, 0:SG], m7T, dsg[:, 0:SG])
                nc.vector.tensor_sub(mu8[:, SG:2 * SG], m7T, mu8[:, 0:SG])
                mu9 = mupool.tile([128, 4 * SG], BF16, tag="mu9")
                for j1 in range(2):
                    nc.vector.tensor_mul(
                        mu9[:, j1 * SG:(j1 + 1) * SG],
                        mu8[:, j1 * SG:(j1 + 1) * SG],
                        dsg[:, (1 + j1) * SG:(2 + j1) * SG],
                    )
                    nc.vector.tensor_sub(
                        mu9[:, (2 + j1) * SG:(3 + j1) * SG],
                        mu8[:, j1 * SG:(j1 + 1) * SG],
                        mu9[:, j1 * SG:(j1 + 1) * SG],
                    )
                # level 9 interleaved with the leaf matmul: each mu10 chunk
                # feeds its accumulation step right away so the PE overlaps
                # the DVE tree instead of waiting for all of it (tail case)
                mu10 = mupool.tile([128, 8 * SG], BF16, tag="mu10")
                yp = py.tile([NCLS, SG], F32, tag="y")

                def leaf_mm(j3):
                    nc.tensor.matmul(
                        yp,
                        pp[:, j3 * NCLS:(j3 + 1) * NCLS],
                        mu10[:, j3 * SG:(j3 + 1) * SG],
                        start=(j3 == 0), stop=(j3 == 7),
                    )

                for j2 in range(4):
                    nc.vector.tensor_mul(
                        mu10[:, j2 * SG:(j2 + 1) * SG],
                        mu9[:, j2 * SG:(j2 + 1) * SG],
                        dsg[:, (3 + j2) * SG:(4 + j2) * SG],
                    )
                    leaf_mm(j2)
                for j2 in range(4):
                    nc.vector.tensor_sub(
                        mu10[:, (4 + j2) * SG:(5 + j2) * SG],
                        mu9[:, j2 * SG:(j2 + 1) * SG],
                        mu10[:, j2 * SG:(j2 + 1) * SG],
                    )
                    leaf_mm(4 + j2)
                ysb = opool.tile([NCLS, SG], F32, tag="ysb")
                nc.scalar.copy(ysb, yp)
                # store on the ACT hwdge queue: it trails the ysb copy in the
                # same FIFO and never delays feature loads on the SP queue
                nc.scalar.dma_start(yT[:, sg * SG:(sg + 1) * SG], ysb)
                scope_pb.__exit__(None, None, None)

            # software pipeline: emit stage2(sg) after stage1(sg+1) so the PE
            # stream never waits on the DVE tree of the previous supergroup.
            prev = None
            ft_cur = ft0
            for sg in range(NSG):
                # prefetch the next sg's features FIRST: these triggers have
                # no compute dependencies beyond the WAR on the tile buffer,
                # which clears a full block earlier
                ft_next = load_ft(sg + 1) if sg + 1 < NSG else None
                st = stage1(sg, ft=ft_cur)
                if prev is not None:
                    stage2(prev)
                prev = st
                ft_cur = ft_next
            stage2(prev)

    nc.finalize()
    return nc


_PROGRAM = None


def _get_program():
    global _PROGRAM
    if _PROGRAM is None:
        _PROGRAM = _build_program()
    return _PROGRAM


def kernel(features, mask, W, b, pi):
    global LAST_RESULT
    features = np.asarray(features, dtype=np.float32)
    mask = np.asarray(mask)
    W = np.asarray(W, dtype=np.float32)
    b = np.asarray(b, dtype=np.float32)
    pi = np.asarray(pi, dtype=np.float32)

    # one-hot selection -> host column gather; apply slot/leaf permutations
    idx = np.argmax(mask, axis=1)
    node = _node_of_slot()
    W2p = W[:, node] * WSCALE
    w2p_resh = np.ascontiguousarray(
        W2p.reshape(KCH, 128, NT, 128).transpose(1, 2, 0, 3).reshape(128, NT * NF)
    )
    w2p_fp8 = np.clip(w2p_resh, -240.0, 240.0).astype(ml_dtypes.float8_e4m3fn)
    b2 = b[node].astype(np.float32)
    bcols = b2.reshape(NT, 128).T                      # [128, NT]
    biases = np.ascontiguousarray(
        np.concatenate([bcols, -bcols], axis=1), dtype=np.float32
    )

    e = np.exp(pi.astype(np.float64) - pi.max(1, keepdims=True))
    probs = (e / e.sum(1, keepdims=True)).astype(np.float32)
    piP = probs[_leaf_of_row(), :]
    pip_resh = np.ascontiguousarray(
        piP.reshape(NT, 128, NCLS).transpose(1, 0, 2).reshape(128, NT * NCLS)
    ).astype(ml_dtypes.bfloat16)
    feat_fp8 = np.clip(features[:, idx], -240.0, 240.0).astype(
        ml_dtypes.float8_e4m3fn
    )

    nc = _get_program()
    in_maps = []
    for c in range(NCORES):
        xc = feat_fp8[c * BC:(c + 1) * BC]            # [BC, NF]
        # device layout [p, sg, k, b]: feat[p, ...] = x[sg*SG+b, 128k+p]
        fdev = np.ascontiguousarray(
            xc.reshape(NSG, SG, KCH, 128).transpose(3, 0, 2, 1).reshape(128, -1)
        )
        in_maps.append(
            {"feat": fdev, "w2p": w2p_fp8, "biases": biases, "pip": pip_resh}
        )
    res = run_bass_kernel_spmd(nc, in_maps, core_ids=list(range(NCORES)), **RUN_KWARGS)
    LAST_RESULT = res
    yT_full = np.concatenate([res.results[c]["yT"] for c in range(NCORES)], axis=1)
    return np.ascontiguousarray(yT_full.T)
